# revision 1
# baseline (speedup 1.0000x reference)
"""Trainium2 Bass kernel for the batched ConstituencyTreeLSTM cell.

Data-parallel across 8 NeuronCores: each core processes 12500 nodes
(padded to 12544 = 98 micro-tiles of 128 nodes). Per 128-node micro-tile:

  leaf:  pre = [e;h_prev]^T, [tag;tagp;1]^T  stationary chunks (5) vs
         fused weight matrix [We;Uh] / [Wt;Wtp;b]  ->  PSUM [128, i|o|fl / u]
         c1 = sig(i)*tanh(u) + sig(fl)*c_prev ; h1 = sig(o)*tanh(c1)
  node:  h1 is transposed on-chip (DMA xbar transpose) into feature-major
         stationary chunks, fused with [tag;tagp;1] and k^T chunks (4) vs
         [Wt_n;Wtp_n;b_n] / Uh_n / Uk_n (gate-permuted, u2 zero-padded)
         c2 = sig(i2)*tanh(u2) + sig(fd2)*q + sig(fl2)*c1
         h2 = sig(o2)*tanh(c2);  out = [h2|c2]

All matmul inputs are bf16 (fp32 PSUM accumulation); biases are folded in
via a ones-row. Elementwise runs bf16 on DVE, activations on ACT.
"""

import os
import sys

import numpy as np

try:
    import concourse.bass as bass  # noqa: F401
except Exception:  # pragma: no cover - fallback for bare environments
    for p in (
        "/root/.axon_site",
        "/root/.axon_site/_ro/trn_rl_repo",
        "/root/.axon_site/_ro/pypackages",
        "/opt/trn_rl_repo",
        "/opt/pypackages",
    ):
        if os.path.isdir(p) and p not in sys.path:
            sys.path.append(p)
    import concourse.bass as bass  # noqa: F401

import ml_dtypes
import concourse.mybir as mybir
import concourse.tile as tile
from concourse import bacc
from concourse.bass_utils import run_bass_kernel_spmd

BF16 = ml_dtypes.bfloat16

N_CORES = 8
N = 100000
NPER = N // N_CORES            # 12500
MICRO = 128                    # nodes per matmul tile (stationary free dim)
MPS = 14                       # micro tiles per supertile
SUPER = MICRO * MPS            # 1792
GRP = 7                        # micro tiles per elementwise batch group
NSUP = 7                       # supertiles per core
NPAD = NSUP * SUPER            # 12544
M = 168                        # mem dim

ROWS_EH = [(0, 128), (128, 256), (256, 384), (384, 468)]  # [e;h_prev] chunks

F32 = mybir.dt.float32
BF = mybir.dt.bfloat16
SIGF = mybir.ActivationFunctionType.Sigmoid
TANHF = mybir.ActivationFunctionType.Tanh

_compiled = None
LAST_RESULT = None


def _build(nsup=NSUP):
    npad = nsup * SUPER
    nc = bacc.Bacc("TRN2", target_bir_lowering=False, debug=False,
                   num_devices=N_CORES)

    xleafT_d = nc.dram_tensor("xleafT", [468, npad], BF, kind="ExternalInput")
    tt1T_d = nc.dram_tensor("tt1T", [101, npad], BF, kind="ExternalInput")
    kT_d = nc.dram_tensor("kT", [168, npad], BF, kind="ExternalInput")
    cq_d = nc.dram_tensor("cq", [npad, 336], BF, kind="ExternalInput")
    w_eh_d = nc.dram_tensor("w_eh", [468, 672], BF, kind="ExternalInput")
    w_ttl_d = nc.dram_tensor("w_ttl", [101, 672], BF, kind="ExternalInput")
    w_ttn_d = nc.dram_tensor("w_ttn", [101, 840], BF, kind="ExternalInput")
    w_b_d = nc.dram_tensor("w_b", [128, 840], BF, kind="ExternalInput")
    w_c_d = nc.dram_tensor("w_c", [128, 840], BF, kind="ExternalInput")
    w_d_d = nc.dram_tensor("w_d", [80, 840], BF, kind="ExternalInput")
    out_d = nc.dram_tensor("out", [npad, 336], BF, kind="ExternalOutput")

    ngrp = 2 * nsup

    with tile.TileContext(nc) as tc:
        from contextlib import ExitStack
        with ExitStack() as ctx:
            wpool = ctx.enter_context(tc.tile_pool(name="w", bufs=1))
            spool = ctx.enter_context(tc.tile_pool(name="s", bufs=2))
            gpool = ctx.enter_context(tc.tile_pool(name="g", bufs=2))
            stpool = ctx.enter_context(tc.tile_pool(name="st", bufs=4))
            plpool = ctx.enter_context(
                tc.tile_pool(name="pl", bufs=2, space="PSUM"))
            pnpool = ctx.enter_context(
                tc.tile_pool(name="pn", bufs=2, space="PSUM"))

            # ---- weights (resident) ----
            wleaf = []
            for i, (r0, r1) in enumerate(ROWS_EH):
                t = wpool.tile([r1 - r0, 672], BF, tag=f"weh{i}")
                nc.sync.dma_start(t[:], w_eh_d[r0:r1, :])
                wleaf.append(t)
            wttl = wpool.tile([101, 672], BF, tag="wttl")
            nc.sync.dma_start(wttl[:], w_ttl_d[:, :])
            wttn = wpool.tile([101, 840], BF, tag="wttn")
            nc.sync.dma_start(wttn[:], w_ttn_d[:, :])
            wb = wpool.tile([128, 840], BF, tag="wb")
            nc.sync.dma_start(wb[:], w_b_d[:, :])
            wc = wpool.tile([128, 840], BF, tag="wc")
            nc.sync.dma_start(wc[:], w_c_d[:, :])
            wd = wpool.tile([80, 840], BF, tag="wd")
            nc.sync.dma_start(wd[:], w_d_d[:, :])

            sstate = {}

            def load_supertile(s):
                cs = s * SUPER
                E = []
                for i, (r0, r1) in enumerate(ROWS_EH):
                    t = spool.tile([r1 - r0, SUPER], BF, tag=f"E{i}")
                    nc.sync.dma_start(t[:], xleafT_d[r0:r1, cs:cs + SUPER])
                    E.append(t)
                TTs = spool.tile([101, SUPER], BF, tag="TT")
                nc.sync.dma_start(TTs[:], tt1T_d[:, cs:cs + SUPER])
                Ct = spool.tile([128, SUPER], BF, tag="C")
                nc.sync.dma_start(Ct[40:128, :], kT_d[0:88, cs:cs + SUPER])
                Dt = spool.tile([80, SUPER], BF, tag="D")
                nc.sync.dma_start(Dt[:], kT_d[88:168, cs:cs + SUPER])
                Bt = spool.tile([128, SUPER], BF, tag="B")
                CQt = spool.tile([128, MPS, 336], BF, tag="CQ")
                nc.sync.dma_start(
                    CQt[:],
                    cq_d[cs:cs + SUPER, :].rearrange("(m p) f -> p m f", p=128))
                OUTt = spool.tile([128, MPS, 336], BF, tag="OUT")
                sstate[s] = dict(E=E, TT=TTs, C=Ct, D=Dt, B=Bt, CQ=CQt,
                                 OUT=OUTt)

            def store_supertile(s):
                cs = s * SUPER
                nc.sync.dma_start(
                    out_d[cs:cs + SUPER, :].rearrange("(m p) f -> p m f",
                                                      p=128),
                    sstate[s]["OUT"][:])

            gstate = {}

            def leaf_block(g):
                st = sstate[g // 2]
                E, TTs = st["E"], st["TT"]
                sgl = gpool.tile([128, GRP, 504], BF, tag="sgl")
                tul = gpool.tile([128, GRP, 168], BF, tag="tul")
                chunks = [(E[0], wleaf[0]), (E[1], wleaf[1]),
                          (E[2], wleaf[2]), (E[3], wleaf[3]), (TTs, wttl)]
                for j in range(GRP):
                    jj = (g % 2) * GRP + j
                    c0 = jj * MICRO
                    P = plpool.tile([128, 1024], F32, tag="psl")
                    for ci, (X, W) in enumerate(chunks):
                        first, last = ci == 0, ci == len(chunks) - 1
                        nc.tensor.matmul(P[:, 0:504], X[:, c0:c0 + MICRO],
                                         W[:, 0:504], start=first, stop=last)
                        nc.tensor.matmul(P[:, 512:680], X[:, c0:c0 + MICRO],
                                         W[:, 504:672], start=first, stop=last)
                    nc.scalar.activation(sgl[:, j, :], P[:, 0:504], SIGF)
                    nc.scalar.activation(tul[:, j, :], P[:, 512:680], TANHF)
                gstate[g] = dict(sgl=sgl, tul=tul)

            def chain_block(g):
                s = g // 2
                st = sstate[s]
                gs = gstate[g]
                sgl, tul = gs["sgl"], gs["tul"]
                CQt, Bt, Ct = st["CQ"], st["B"], st["C"]
                g7 = (g % 2) * GRP
                t1 = gpool.tile([128, GRP, 168], BF, tag="tmpA")
                nc.vector.tensor_mul(t1[:], sgl[:, :, 0:168], tul[:])
                t2 = gpool.tile([128, GRP, 168], BF, tag="tmpB")
                nc.vector.tensor_mul(t2[:], sgl[:, :, 336:504],
                                     CQt[:, g7:g7 + GRP, 0:168])
                c1t = gpool.tile([128, GRP, 168], BF, tag="c1")
                nc.vector.tensor_add(c1t[:], t1[:], t2[:])
                tc1 = gpool.tile([128, GRP, 168], BF, tag="tc1")
                nc.scalar.activation(tc1[:], c1t[:], TANHF)
                h1t = gpool.tile([128, GRP, 256], BF, tag="h1")
                nc.gpsimd.memset(h1t[:, :, 168:256], 0.0)
                nc.vector.tensor_mul(h1t[:, :, 0:168], sgl[:, :, 168:336],
                                     tc1[:])
                for j in range(GRP):
                    jj = (g % 2) * GRP + j
                    c0 = jj * MICRO
                    nc.scalar.dma_start_transpose(Bt[:, c0:c0 + MICRO],
                                                  h1t[:, j, 0:128])
                    stg = stpool.tile([128, 128], BF, tag="stg")
                    nc.scalar.dma_start_transpose(stg[:], h1t[:, j, 128:256])
                    nc.vector.tensor_copy(Ct[0:40, c0:c0 + MICRO],
                                          stg[0:40, :])
                gstate[g]["c1"] = c1t

            def node_block(g):
                s = g // 2
                st = sstate[s]
                TTs, Bt, Ct, Dt = st["TT"], st["B"], st["C"], st["D"]
                CQt, OUTt = st["CQ"], st["OUT"]
                c1t = gstate[g]["c1"]
                g7 = (g % 2) * GRP
                sgn = gpool.tile([128, GRP, 672], BF, tag="sgn")
                tu2 = gpool.tile([128, GRP, 168], BF, tag="tu2")
                chunks = [(TTs, wttn), (Bt, wb), (Ct, wc), (Dt, wd)]
                for j in range(GRP):
                    jj = g7 + j
                    c0 = jj * MICRO
                    P = pnpool.tile([128, 1024], F32, tag="psn")
                    for ci, (X, W) in enumerate(chunks):
                        first, last = ci == 0, ci == len(chunks) - 1
                        nc.tensor.matmul(P[:, 0:504], X[:, c0:c0 + MICRO],
                                         W[:, 0:504], start=first, stop=last)
                        nc.tensor.matmul(P[:, 512:848], X[:, c0:c0 + MICRO],
                                         W[:, 504:840], start=first, stop=last)
                    pr = P[:].rearrange("p (a b) -> p a b", a=2, b=512)
                    sr = sgn[:, j, :].rearrange("p (a b) -> p a b", a=2, b=336)
                    nc.scalar.activation(sr, pr[:, :, 0:336], SIGF)
                    nc.scalar.activation(tu2[:, j, :], P[:, 336:504], TANHF)
                t3 = gpool.tile([128, GRP, 168], BF, tag="tmpA")
                nc.vector.tensor_mul(t3[:], sgn[:, :, 0:168], tu2[:])
                t4 = gpool.tile([128, GRP, 168], BF, tag="tmpB")
                nc.vector.tensor_mul(t4[:], sgn[:, :, 504:672],
                                     CQt[:, g7:g7 + GRP, 168:336])
                t5 = gpool.tile([128, GRP, 168], BF, tag="tmpC")
                nc.vector.tensor_mul(t5[:], sgn[:, :, 336:504], c1t[:])
                t6 = gpool.tile([128, GRP, 168], BF, tag="tmpD")
                nc.vector.tensor_add(t6[:], t3[:], t4[:])
                nc.vector.tensor_add(OUTt[:, g7:g7 + GRP, 168:336], t6[:],
                                     t5[:])
                tc2 = gpool.tile([128, GRP, 168], BF, tag="tc2")
                nc.scalar.activation(tc2[:], OUTt[:, g7:g7 + GRP, 168:336],
                                     TANHF)
                nc.vector.tensor_mul(OUTt[:, g7:g7 + GRP, 0:168],
                                     sgn[:, :, 168:336], tc2[:])

            for g in range(ngrp):
                if g % 2 == 0:
                    s = g // 2
                    if s == 0:
                        load_supertile(0)
                    if s + 1 < nsup:
                        load_supertile(s + 1)
                leaf_block(g)
                if g >= 1:
                    node_block(g - 1)
                    if (g - 1) % 2 == 1:
                        store_supertile((g - 1) // 2)
                chain_block(g)
            node_block(ngrp - 1)
            store_supertile(nsup - 1)

    nc.compile()
    return nc


def _prep_core(inputs, c, npad=NPAD, nper=NPER):
    """Build the per-core (sharded, transposed, bf16) input arrays."""
    sl = slice(c * nper, (c + 1) * nper)
    e = inputs["e"][sl]
    h_prev = inputs["h_prev"][sl]
    tag = inputs["tag"][sl]
    tagp = inputs["tag_parent"][sl]
    k = inputs["k"][sl]
    c_prev = inputs["c_prev"][sl]
    q = inputs["q"][sl]
    n = e.shape[0]

    xleafT = np.zeros((468, npad), BF16)
    xleafT[0:300, :n] = e.T
    xleafT[300:468, :n] = h_prev.T
    tt1T = np.zeros((101, npad), BF16)
    tt1T[0:50, :n] = tag.T
    tt1T[50:100, :n] = tagp.T
    tt1T[100, :n] = 1.0
    kT = np.zeros((168, npad), BF16)
    kT[:, :n] = k.T
    cq = np.zeros((npad, 336), BF16)
    cq[:n, 0:168] = c_prev
    cq[:n, 168:336] = q
    return dict(xleafT=xleafT, tt1T=tt1T, kT=kT, cq=cq)


def _prep_weights(inputs):
    cat = np.concatenate
    w_eh = cat([inputs["We_l"], inputs["Uh_l"]], 0).astype(BF16)
    w_ttl = cat([inputs["Wt_l"], inputs["Wtp_l"], inputs["b_l"][None, :]],
                0).astype(BF16)
    # node gate order: [i2, o2, u2 | fl2, fd2] (source order i,o,fl,fd,u)
    perm = np.concatenate([np.arange(0, 336), np.arange(672, 840),
                           np.arange(336, 672)])
    w_ttn = cat([inputs["Wt_n"], inputs["Wtp_n"], inputs["b_n"][None, :]],
                0)[:, perm].astype(BF16)
    uh = inputs["Uh_n"][:, perm].astype(BF16)
    uk = np.zeros((168, 840), BF16)
    uk[:, 0:336] = inputs["Uk_n"][:, 0:336]      # i2, o2
    uk[:, 504:672] = inputs["Uk_n"][:, 336:504]  # fl2
    uk[:, 672:840] = inputs["Uk_n"][:, 504:672]  # fd2
    w_b = np.ascontiguousarray(uh[0:128])
    w_c = cat([uh[128:168], uk[0:88]], 0)
    w_d = np.ascontiguousarray(uk[88:168])
    return dict(w_eh=w_eh, w_ttl=w_ttl, w_ttn=w_ttn, w_b=w_b, w_c=w_c,
                w_d=w_d)


def kernel(**inputs):
    global _compiled, LAST_RESULT
    if _compiled is None:
        _compiled = _build()
    weights = _prep_weights(inputs)
    in_maps = []
    for c in range(N_CORES):
        m = _prep_core(inputs, c)
        m.update(weights)
        in_maps.append(m)
    res = run_bass_kernel_spmd(_compiled, in_maps,
                               core_ids=list(range(N_CORES)))
    LAST_RESULT = res
    outs = [res.results[c]["out"][:NPER].astype(np.float32)
            for c in range(N_CORES)]
    return np.concatenate(outs, 0)


# revision 9
# speedup vs baseline: 1.7229x; 1.7229x over previous
"""Trainium2 Bass kernel for the batched ConstituencyTreeLSTM cell.

Data-parallel across 8 NeuronCores: each core processes 12500 nodes
(padded to 12544 = 98 micro-tiles of 128 nodes, grouped 7 per "group").
Per 128-node micro-tile:

  leaf:  stationary chunks [e;h_prev]^T (4) + [tag;tagp;1]^T (1) vs fused
         weights [We;Uh] / [Wt;Wtp;b]  ->  bf16 PSUM [128, i|o|fl|u]
         c1 = sig(i)*tanh(u) + sig(fl)*c_prev ; h1 = sig(o)*tanh(c1)
  node:  h1^T obtained on-chip (rows 0:128 via DMA xbar transpose, rows
         128:168 via PE transpose); stationary chunks [tag;tagp;1]^T,
         h1^T, [h1^T;k^T], k^T (4) vs [Wt_n;Wtp_n;b_n] / Uh_n / Uk_n
         (u2 gate zero-padded in Uk)  ->  bf16 PSUM [128, i2|o2|fl2|fd2|u2]
         c2 = sig(i2)*tanh(u2) + sig(fd2)*q + sig(fl2)*c1
         h2 = sig(o2)*tanh(c2);  out = [h2|c2]

All matmul inputs are bf16 (bf16 PSUM accumulation); biases are folded in
via a ones-row. Elementwise runs bf16 on DVE, activations on ACT.
Pipeline: node block of group g runs two iterations after its leaf block
so the h1 -> transpose chain never stalls the TensorEngine (keeps HAM warm).
"""

import os
import sys

import numpy as np

try:
    import concourse.bass as bass  # noqa: F401
except Exception:  # pragma: no cover - fallback for bare environments
    for p in (
        "/root/.axon_site",
        "/root/.axon_site/_ro/trn_rl_repo",
        "/root/.axon_site/_ro/pypackages",
        "/opt/trn_rl_repo",
        "/opt/pypackages",
    ):
        if os.path.isdir(p) and p not in sys.path:
            sys.path.append(p)
    import concourse.bass as bass  # noqa: F401

import ml_dtypes
import concourse.mybir as mybir
import concourse.tile as tile
from concourse import bacc
from concourse.bass_utils import run_bass_kernel_spmd
from concourse.masks import make_identity

BF16 = ml_dtypes.bfloat16

N_CORES = 8
N = 100000
NPER = N // N_CORES            # 12500
MICRO = 128                    # nodes per matmul tile (stationary free dim)
GRP = 7                        # micro tiles per group
GNODES = MICRO * GRP           # 896
NGRP = 14                      # groups per core
NPAD = NGRP * GNODES           # 12544
M = 168                        # mem dim

ROWS_EH = [(0, 128), (128, 256), (256, 384), (384, 468)]  # [e;h_prev] chunks

F32 = mybir.dt.float32
BF = mybir.dt.bfloat16
SIGF = mybir.ActivationFunctionType.Sigmoid
TANHF = mybir.ActivationFunctionType.Tanh

_compiled = None
LAST_RESULT = None


def _build(ngrp=NGRP):
    npad = ngrp * GNODES
    nc = bacc.Bacc("TRN2", target_bir_lowering=False, debug=False,
                   num_devices=N_CORES)

    xleafT_d = nc.dram_tensor("xleafT", [468, npad], BF, kind="ExternalInput")
    tt1T_d = nc.dram_tensor("tt1T", [101, npad], BF, kind="ExternalInput")
    kT_d = nc.dram_tensor("kT", [168, npad], BF, kind="ExternalInput")
    cq_d = nc.dram_tensor("cq", [npad, 336], BF, kind="ExternalInput")
    w_eh_d = nc.dram_tensor("w_eh", [468, 672], BF, kind="ExternalInput")
    w_ttl_d = nc.dram_tensor("w_ttl", [101, 672], BF, kind="ExternalInput")
    w_ttn_d = nc.dram_tensor("w_ttn", [101, 840], BF, kind="ExternalInput")
    w_b_d = nc.dram_tensor("w_b", [128, 840], BF, kind="ExternalInput")
    w_c_d = nc.dram_tensor("w_c", [128, 840], BF, kind="ExternalInput")
    w_d_d = nc.dram_tensor("w_d", [80, 840], BF, kind="ExternalInput")
    out_d = nc.dram_tensor("out", [npad, 336], BF, kind="ExternalOutput")

    with tile.TileContext(nc) as tc:
        from contextlib import ExitStack
        with ExitStack() as ctx:
            wpool = ctx.enter_context(tc.tile_pool(name="w", bufs=1))
            spool = ctx.enter_context(tc.tile_pool(name="s", bufs=4))
            opool = ctx.enter_context(tc.tile_pool(name="o", bufs=2))
            gpool = ctx.enter_context(tc.tile_pool(name="g", bufs=2))
            c1pool = ctx.enter_context(tc.tile_pool(name="c1p", bufs=3))
            plpool = ctx.enter_context(
                tc.tile_pool(name="pl", bufs=2, space="PSUM"))
            pnpool = ctx.enter_context(
                tc.tile_pool(name="pn", bufs=2, space="PSUM"))

            # ---- constants / weights (resident) ----
            ident = wpool.tile([128, 128], BF, tag="ident")
            make_identity(nc, ident[:])
            wleaf = []
            for i, (r0, r1) in enumerate(ROWS_EH):
                t = wpool.tile([r1 - r0, 672], BF, tag=f"weh{i}")
                nc.sync.dma_start(t[:], w_eh_d[r0:r1, :])
                wleaf.append(t)
            wttl = wpool.tile([101, 672], BF, tag="wttl")
            nc.sync.dma_start(wttl[:], w_ttl_d[:, :])
            wttn = wpool.tile([101, 840], BF, tag="wttn")
            nc.sync.dma_start(wttn[:], w_ttn_d[:, :])
            wb = wpool.tile([128, 840], BF, tag="wb")
            nc.sync.dma_start(wb[:], w_b_d[:, :])
            wc = wpool.tile([128, 840], BF, tag="wc")
            nc.sync.dma_start(wc[:], w_c_d[:, :])
            wd = wpool.tile([80, 840], BF, tag="wd")
            nc.sync.dma_start(wd[:], w_d_d[:, :])

            sstate = {}
            gstate = {}

            def load_group(g):
                cs = g * GNODES
                E = []
                for i, (r0, r1) in enumerate(ROWS_EH):
                    t = spool.tile([r1 - r0, GNODES], BF, tag=f"E{i}")
                    nc.sync.dma_start(t[:], xleafT_d[r0:r1, cs:cs + GNODES])
                    E.append(t)
                TTs = spool.tile([101, GNODES], BF, tag="TT")
                nc.sync.dma_start(TTs[:], tt1T_d[:, cs:cs + GNODES])
                Ct = spool.tile([128, GNODES], BF, tag="C")
                nc.sync.dma_start(Ct[40:128, :], kT_d[0:88, cs:cs + GNODES])
                Dt = spool.tile([80, GNODES], BF, tag="D")
                nc.sync.dma_start(Dt[:], kT_d[88:168, cs:cs + GNODES])
                Bt = spool.tile([128, GNODES], BF, tag="B")
                CQt = spool.tile([128, GRP, 336], BF, tag="CQ")
                nc.sync.dma_start(
                    CQt[:],
                    cq_d[cs:cs + GNODES, :].rearrange("(m p) f -> p m f",
                                                      p=128))
                sstate[g] = dict(E=E, TT=TTs, C=Ct, D=Dt, B=Bt, CQt=CQt)

            def leaf_block(g):
                st = sstate[g]
                E, TTs = st["E"], st["TT"]
                sgl = gpool.tile([128, GRP, 504], BF, tag="sgl")
                tul = gpool.tile([128, GRP, 168], BF, tag="tul")
                chunks = [(E[0], wleaf[0]), (E[1], wleaf[1]),
                          (E[2], wleaf[2]), (E[3], wleaf[3]), (TTs, wttl)]
                for j in range(GRP):
                    c0 = j * MICRO
                    P = plpool.tile([128, 1024], F32, tag="psl")
                    for ci, (X, W) in enumerate(chunks):
                        first, last = ci == 0, ci == len(chunks) - 1
                        nc.tensor.matmul(P[:, 0:504], X[:, c0:c0 + MICRO],
                                         W[:, 0:504], start=first, stop=last)
                        nc.tensor.matmul(P[:, 512:680], X[:, c0:c0 + MICRO],
                                         W[:, 504:672], start=first, stop=last)
                    nc.scalar.activation(sgl[:, j, :], P[:, 0:504], SIGF)
                    nc.scalar.activation(tul[:, j, :], P[:, 512:680], TANHF)
                gstate[g] = dict(sgl=sgl, tul=tul)

            def chain_block(g):
                st = sstate[g]
                gs = gstate[g]
                sgl, tul = gs["sgl"], gs["tul"]
                CQt, Bt, Ct = st["CQt"], st["B"], st["C"]
                t1 = gpool.tile([128, GRP, 168], BF, tag="tmpA")
                nc.vector.tensor_mul(t1[:], sgl[:, :, 0:168], tul[:])
                t2 = gpool.tile([128, GRP, 168], BF, tag="tmpB")
                nc.vector.tensor_mul(t2[:], sgl[:, :, 336:504],
                                     CQt[:, :, 0:168])
                c1t = c1pool.tile([128, GRP, 168], BF, tag="c1")
                nc.vector.tensor_add(c1t[:], t1[:], t2[:])
                tc1 = gpool.tile([128, GRP, 168], BF, tag="tc1")
                nc.scalar.activation(tc1[:], c1t[:], TANHF)
                h1t = c1pool.tile([128, GRP, 168], BF, tag="h1")
                nc.vector.tensor_mul(h1t[:], sgl[:, :, 168:336], tc1[:])
                for j in range(GRP):
                    c0 = j * MICRO
                    nc.sync.dma_start_transpose(Bt[:, c0:c0 + MICRO],
                                                h1t[:, j, 0:128])
                gs["c1"] = c1t
                gs["h1"] = h1t

            def node_block(g):
                st = sstate[g]
                TTs, Bt, Ct, Dt = st["TT"], st["B"], st["C"], st["D"]
                CQt = st["CQt"]
                c1t = gstate[g]["c1"]
                h1t = gstate[g]["h1"]
                OUTt = opool.tile([128, GRP, 336], BF, tag="OUT")
                sgn = gpool.tile([128, GRP, 672], BF, tag="sgn")
                tu2 = gpool.tile([128, GRP, 168], BF, tag="tu2")
                chunks = [(TTs, wttn), (Bt, wb), (Ct, wc), (Dt, wd)]
                for j in range(GRP):
                    c0 = j * MICRO
                    P = pnpool.tile([128, 1024], F32, tag="psn")
                    # 40-row h1 tail transpose into bank-1 spare space,
                    # evacuated into the C chunk before the matmuls start.
                    pt = P[0:40, 896:960].bitcast(BF)
                    nc.tensor.transpose(pt, h1t[:, j, 128:168], ident[:])
                    nc.vector.tensor_copy(Ct[0:40, c0:c0 + MICRO], pt)
                    for ci, (X, W) in enumerate(chunks):
                        first, last = ci == 0, ci == len(chunks) - 1
                        nc.tensor.matmul(P[:, 0:504], X[:, c0:c0 + MICRO],
                                         W[:, 0:504], start=first, stop=last)
                        nc.tensor.matmul(P[:, 512:848], X[:, c0:c0 + MICRO],
                                         W[:, 504:840], start=first, stop=last)
                    pr = P[:].rearrange("p (a b) -> p a b", a=2, b=512)
                    sr = sgn[:, j, :].rearrange("p (a b) -> p a b", a=2, b=336)
                    nc.scalar.activation(sr, pr[:, :, 0:336], SIGF)
                    nc.scalar.activation(tu2[:, j, :], P[:, 336:504], TANHF)
                t3 = gpool.tile([128, GRP, 168], BF, tag="tmpA")
                nc.vector.tensor_mul(t3[:], sgn[:, :, 0:168], tu2[:])
                t4 = gpool.tile([128, GRP, 168], BF, tag="tmpB")
                nc.vector.tensor_mul(t4[:], sgn[:, :, 504:672],
                                     CQt[:, :, 168:336])
                t5 = gpool.tile([128, GRP, 168], BF, tag="tmpC")
                nc.vector.tensor_mul(t5[:], sgn[:, :, 336:504], c1t[:])
                t6 = gpool.tile([128, GRP, 168], BF, tag="tmpA")
                nc.vector.tensor_add(t6[:], t3[:], t4[:])
                nc.vector.tensor_add(OUTt[:, :, 168:336], t6[:], t5[:])
                tc2 = gpool.tile([128, GRP, 168], BF, tag="tc2")
                nc.scalar.activation(tc2[:], OUTt[:, :, 168:336], TANHF)
                nc.vector.tensor_mul(OUTt[:, :, 0:168],
                                     sgn[:, :, 168:336], tc2[:])
                return OUTt

            def store_group(g, OUTt):
                cs = g * GNODES
                nc.gpsimd.dma_start(
                    out_d[cs:cs + GNODES, :].rearrange("(m p) f -> p m f",
                                                       p=128),
                    OUTt[:])

            load_group(0)
            if ngrp > 1:
                load_group(1)
            for g in range(ngrp):
                leaf_block(g)
                if g >= 2:
                    store_group(g - 2, node_block(g - 2))
                if g + 2 < ngrp:
                    load_group(g + 2)
                chain_block(g)
            for g in (ngrp - 2, ngrp - 1):
                if g >= 0:
                    store_group(g, node_block(g))

    nc.compile()
    return nc


def _prep_core(inputs, c, npad=NPAD, nper=NPER):
    """Build the per-core (sharded, transposed, bf16) input arrays."""
    sl = slice(c * nper, (c + 1) * nper)
    e = inputs["e"][sl]
    h_prev = inputs["h_prev"][sl]
    tag = inputs["tag"][sl]
    tagp = inputs["tag_parent"][sl]
    k = inputs["k"][sl]
    c_prev = inputs["c_prev"][sl]
    q = inputs["q"][sl]
    n = e.shape[0]

    xleafT = np.zeros((468, npad), BF16)
    xleafT[0:300, :n] = e.T
    xleafT[300:468, :n] = h_prev.T
    tt1T = np.zeros((101, npad), BF16)
    tt1T[0:50, :n] = tag.T
    tt1T[50:100, :n] = tagp.T
    tt1T[100, :n] = 1.0
    kT = np.zeros((168, npad), BF16)
    kT[:, :n] = k.T
    cq = np.zeros((npad, 336), BF16)
    cq[:n, 0:168] = c_prev
    cq[:n, 168:336] = q
    return dict(xleafT=xleafT, tt1T=tt1T, kT=kT, cq=cq)


def _prep_weights(inputs):
    cat = np.concatenate
    w_eh = cat([inputs["We_l"], inputs["Uh_l"]], 0).astype(BF16)
    w_ttl = cat([inputs["Wt_l"], inputs["Wtp_l"], inputs["b_l"][None, :]],
                0).astype(BF16)
    # node gate order: [i2, o2, u2 | fl2, fd2] (source order i,o,fl,fd,u)
    perm = np.concatenate([np.arange(0, 336), np.arange(672, 840),
                           np.arange(336, 672)])
    w_ttn = cat([inputs["Wt_n"], inputs["Wtp_n"], inputs["b_n"][None, :]],
                0)[:, perm].astype(BF16)
    uh = inputs["Uh_n"][:, perm].astype(BF16)
    uk = np.zeros((168, 840), BF16)
    uk[:, 0:336] = inputs["Uk_n"][:, 0:336]      # i2, o2
    uk[:, 504:672] = inputs["Uk_n"][:, 336:504]  # fl2
    uk[:, 672:840] = inputs["Uk_n"][:, 504:672]  # fd2
    w_b = np.ascontiguousarray(uh[0:128])
    w_c = cat([uh[128:168], uk[0:88]], 0)
    w_d = np.ascontiguousarray(uk[88:168])
    return dict(w_eh=w_eh, w_ttl=w_ttl, w_ttn=w_ttn, w_b=w_b, w_c=w_c,
                w_d=w_d)


def kernel(**inputs):
    global _compiled, LAST_RESULT
    if _compiled is None:
        _compiled = _build()
    weights = _prep_weights(inputs)
    in_maps = []
    for c in range(N_CORES):
        m = _prep_core(inputs, c)
        m.update(weights)
        in_maps.append(m)
    res = run_bass_kernel_spmd(_compiled, in_maps,
                               core_ids=list(range(N_CORES)))
    LAST_RESULT = res
    outs = [res.results[c]["out"][:NPER].astype(np.float32)
            for c in range(N_CORES)]
    return np.concatenate(outs, 0)
